# revision 30
# baseline (speedup 1.0000x reference)
"""Trainium2 Bass kernel: 16-head attention with RoPE (dense_transformer).

Sharding: tensor-parallel over heads. 8 cores x 2 heads each.
Each core: Wq/Wk/Wv column slice [1024,128], Wo row slice [128,1024],
full input; computes its heads' attention + partial output projection.
Host sums the 8 partial outputs (row-parallel Wo reduction) and adds bo.

Device layout is "transposed": Q^T/K^T/V^T/ctx^T are kept as [dim, seq]
with the head dim on SBUF partitions, so projections come straight out
of the PE at free=512, scores^T = K^T.T @ Q^T needs no transposes, and
the softmax denominator falls out of an extra ones-column in V.

v7 schedule: the kernel is paced by the softmax exp (only ACT has an exp
LUT; 128 FD=1024 exps ~ 143us busy). Everything else hides under it:
 - xt arrives POSITION-major (8 chunks of 512 positions x all channels);
   Q/K projection + rope of stripe 0 complete right after chunk 0 lands.
   Queue plumbing: chunks 0-3 + rope swaps on sync/gpsimd (swaps never
   behind a 1MB chunk), bulk stream (weights + chunks 4-7) on scalar.
 - rope per-512-stripe: tensor_scalar_add stages the biased plain,
   scalar_tensor_tensor fuses (ps+b)*cos from PSUM. ACT only does exp.
 - V is projected TRANSPOSED (8 MMs of free=512 per stripe, like Q/K)
   then flipped per 128-key block by PE transpose-mode against an
   identity (a chain of 128-free matmuls costs ~4x more than
   stripe-proj + transpose).
 - V columns per key block: [ones | 0*63 | h0 dk | h1 dk | ones], so
   head0 ctx lands on PSUM rows 64:128 (M=128 lhsT) and head1 ctx on
   rows 0:64 (M=65 lhsT) - both evacuate with lane-aligned copies, no
   shuffles; Wo rows are permuted host-side to ctxT rows = [h1|h0].
   Dens land on PSUM rows 0 (h0) and 64 (h1).
 - softmax division per-stripe, staged as separate pump units so the
   cross-engine latency chain never head-of-line blocks a queue:
   evac triggers den->DRAM->[128,8]; a later unit runs the reciprocal
   (per-lane multi-pass: [1,512] costs 3.3us, [128,8] ~0.2us) and
   returns it to DRAM; a later unit DMA-broadcasts R tiles; a later
   unit scales the stripe; then the out-projection quads stream out.
 - the last block's ctx chains ride its own exp stream (injected at
   tt 4/8/12) so the tail is one quarter-chain + div + op + DMA.
"""

import sys

if "/opt/trn_rl_repo" not in sys.path:
    sys.path.insert(0, "/opt/trn_rl_repo")

from collections import deque

import numpy as np
import ml_dtypes

B = 2
S = 2048
NS = B * S  # 4096
D = 1024
H = 16
DK = 64
NCORES = 8
HPC = H // NCORES  # heads per core = 2
DPC = HPC * DK  # model dims per core = 128

_cache = {}


def _build_nc():
    import concourse.bass as bass
    import concourse.tile as tile
    import concourse.mybir as mybir
    from concourse import bacc

    fp32 = mybir.dt.float32
    bf16 = mybir.dt.bfloat16
    Exp = mybir.ActivationFunctionType.Exp
    Add = mybir.AluOpType.add
    Mult = mybir.AluOpType.mult

    nc = bacc.Bacc("TRN2", debug=False, num_devices=NCORES)

    xt = nc.dram_tensor("xt", [D, NS], bf16, kind="ExternalInput").ap()
    wq = nc.dram_tensor("wq", [128, 8 * 128], bf16, kind="ExternalInput").ap()
    wk = nc.dram_tensor("wk", [128, 8 * 128], bf16, kind="ExternalInput").ap()
    wv = nc.dram_tensor("wv", [128, 8 * 128], bf16, kind="ExternalInput").ap()
    wo = nc.dram_tensor("wo", [DPC, D], bf16, kind="ExternalInput").ap()
    bq = nc.dram_tensor("bq", [DPC, 1], fp32, kind="ExternalInput").ap()
    bk = nc.dram_tensor("bk", [DPC, 1], fp32, kind="ExternalInput").ap()
    bv = nc.dram_tensor("bv", [DPC, 1], fp32, kind="ExternalInput").ap()
    ident = nc.dram_tensor("ident", [128, 128], bf16, kind="ExternalInput").ap()
    cos_d = nc.dram_tensor("cos", [128, S], bf16, kind="ExternalInput").ap()
    sin_d = nc.dram_tensor("sin", [128, S], bf16, kind="ExternalInput").ap()
    out_d = nc.dram_tensor("out", [D, NS], bf16, kind="ExternalOutput").ap()

    # v_sb columns per key block tt: [0]=ones(h0), [1:64]=0, [64:128]=h0
    # dk, [128:192]=h1 dk, [192]=ones(h1). h0 lhsT = cols 0:128 (M=128,
    # den->row 0, ctx->rows 64:128); h1 lhsT = cols 128:193 (M=65,
    # ctx->rows 0:64, den->row 64).
    VW = 193

    with tile.TileContext(nc) as tc:
        with (
            tc.tile_pool(name="persist", bufs=1) as persist,
            tc.tile_pool(name="stage", bufs=1) as stage,
            tc.tile_pool(name="dram", bufs=1, space="DRAM") as dram,
            tc.tile_pool(name="sc_ps", bufs=2, space="PSUM") as sc_ps,
            tc.tile_pool(name="ctx_ps", bufs=2, space="PSUM") as ctx_ps,
            tc.tile_pool(name="op_ps", bufs=2, space="PSUM") as op_ps,
        ):
            qrot = persist.tile([128, NS], bf16, tag="qrot")
            krot = persist.tile([128, NS], bf16, tag="krot")
            v_sb = persist.tile([128, 32, VW], bf16, tag="v")
            ctxT = persist.tile([128, NS], bf16, tag="ctxT")
            wo_sb = persist.tile([128, 8, 128], bf16, tag="wo")
            cos_sb = persist.tile([128, S], bf16, tag="cos")
            sin_sb = persist.tile([128, S], bf16, tag="sin")
            wq_sb = persist.tile([128, 8, 128], bf16, tag="wq")
            wk_sb = persist.tile([128, 8, 128], bf16, tag="wk")
            wv_sb = persist.tile([128, 8, 128], bf16, tag="wv")
            bq_sb = persist.tile([128, 1], fp32, tag="bq")
            bk_sb = persist.tile([128, 1], fp32, tag="bk")
            bv_sb = persist.tile([128, 1], fp32, tag="bv")
            id_sb = persist.tile([128, 128], bf16, tag="ident")
            xt_sb = persist.tile([128, 8, NS], bf16, tag="xt")
            den_dram = dram.tile([8, 2, 512], bf16, tag="den_dram")
            rc_dram = dram.tile([8, 2, 512], bf16, tag="rc_dram")
            xt_r = xt.rearrange("(c p) s -> p c s", p=128)
            out_r = out_d.rearrange("(j p) s -> p j s", p=128)

            def chunk_dma(eng, s):
                eng.dma_start(
                    xt_sb[:, :, s * 512 : (s + 1) * 512],
                    xt_r[:, :, s * 512 : (s + 1) * 512],
                )

            # --- DMA prologue -------------------------------------------
            nc.scalar.dma_start(wq_sb[:], wq.rearrange("p (c m) -> p c m", m=128))
            nc.scalar.dma_start(bq_sb[:], bq)
            nc.scalar.dma_start(bk_sb[:], bk)
            nc.scalar.dma_start(bv_sb[:], bv)
            nc.scalar.dma_start(cos_sb[:], cos_d)
            nc.scalar.dma_start(sin_sb[:], sin_d)
            nc.sync.dma_start(wk_sb[:], wk.rearrange("p (c m) -> p c m", m=128))
            chunk_dma(nc.sync, 0)
            chunk_dma(nc.gpsimd, 1)

            nc.vector.memset(v_sb[:, :, 0:1], 1.0)
            nc.vector.memset(v_sb[:, :, 1:64], 0.0)
            nc.vector.memset(v_sb[:, :, 192:193], 1.0)

            # --- stripe-level units -------------------------------------
            def proj_stripe(w_sb, s, name):
                ps = op_ps.tile([128, 512], fp32, tag="op", name=f"ps_{name}{s}")
                for ch in range(8):
                    nc.tensor.matmul(
                        ps[:],
                        w_sb[:, ch, :],
                        xt_sb[:, ch, s * 512 : (s + 1) * 512],
                        start=(ch == 0),
                        stop=(ch == 7),
                    )
                return ps

            def rope_stripe(ps, b_sb, s, dst):
                # dst[:, sl] = plain*cos + swap(plain)*sin, plain = ps + b.
                sl = slice(s * 512, (s + 1) * 512)
                so = (s % 4) * 512
                cs = cos_sb[:, so : so + 512]
                sn = sin_sb[:, so : so + 512]
                plain = stage.tile(
                    [128, 512], bf16, tag="plain", bufs=2, name=f"pl{s}"
                )
                nc.vector.tensor_scalar_add(plain[:], ps[:], b_sb[:])
                nc.vector.scalar_tensor_tensor(dst[:, sl], ps[:], b_sb[:], cs, Add, Mult)
                swap = stage.tile(
                    [128, 512], bf16, tag="swap", bufs=2, name=f"sw{s}"
                )
                for g in (0, 64):
                    nc.sync.dma_start(swap[g : g + 32, :], plain[g + 32 : g + 64, :])
                    nc.gpsimd.dma_start(swap[g + 32 : g + 64, :], plain[g : g + 32, :])
                t2 = stage.tile([128, 512], bf16, tag="t2", bufs=2, name=f"t2{s}")
                nc.vector.tensor_mul(t2[:], swap[:], sn)
                nc.vector.tensor_add(dst[:, sl], dst[:, sl], t2[:])

            def qk_unit(w_sb, b_sb, s, dst, name):
                def unit():
                    ps = proj_stripe(w_sb, s, name)
                    rope_stripe(ps, b_sb, s, dst)

                return unit

            vt_sb = {}

            def vproj_unit(s):
                def unit():
                    ps = proj_stripe(wv_sb, s, "v")
                    vt = stage.tile(
                        [128, 512], bf16, tag="vt", bufs=2, name=f"vt{s}"
                    )
                    vt_sb[s] = vt
                    nc.vector.tensor_scalar_add(vt[:], ps[:], bv_sb[:])

                return unit

            def vtrans_unit(tt):
                # PE transpose-mode flips vT [dpc, pos128] to [pos, dpc]
                def unit():
                    vt = vt_sb[tt // 4]
                    pst = op_ps.tile(
                        [128, 128], bf16, tag="op", name=f"pst{tt}"
                    )
                    nc.tensor.transpose(
                        pst[:], vt[:, (tt % 4) * 128 : (tt % 4 + 1) * 128], id_sb[:]
                    )
                    nc.vector.tensor_copy(v_sb[:, tt, 64:192], pst[:])

                return unit

            # --- attention block ----------------------------------------
            work = deque()

            def pump(n):
                for _ in range(n):
                    if work:
                        work.popleft()()

            rds = {}

            def ctx_units(b, sh, si, expS):
                # two 16-matmul ctx chains (head 0 / head 1) for the
                # 512-col stripe, as 8 pump units of 4 MMs + evacuation.
                st_i = sh * 2 + si
                st = b * 4 + st_i
                pcs = {}

                def chain_quarter(h, q):
                    def unit():
                        if q == 0:
                            pcs[h] = ctx_ps.tile(
                                [128 if h == 0 else 65, 512], fp32, tag="pc",
                                name=f"pc{b}{st_i}{h}",
                            )
                        lo = h * 128
                        hi = 128 if h == 0 else 193
                        for tt in range(q * 4, q * 4 + 4):
                            nc.tensor.matmul(
                                pcs[h][:],
                                v_sb[:, b * 16 + tt, lo:hi],
                                expS[:, tt, h * 512 : (h + 1) * 512],
                                start=(tt == 0),
                                stop=(tt == 15),
                            )

                    return unit

                def evacuate():
                    # h0: den row 0, ctx rows 64:128; h1: ctx rows 0:64,
                    # den row 64. All copies lane-aligned. Kick off the
                    # den DRAM bounce immediately; later units consume it.
                    ds0 = b * S + st_i * 512
                    dsb = stage.tile(
                        [128, 512], bf16, tag="dsb", bufs=2, name=f"dsb{st}"
                    )
                    nc.vector.tensor_copy(
                        ctxT[DK:128, ds0 : ds0 + 512], pcs[0][DK:128, :]
                    )
                    nc.vector.tensor_copy(dsb[0:1, :], pcs[0][0:1, :])
                    nc.vector.tensor_copy(
                        ctxT[0:DK, ds0 : ds0 + 512], pcs[1][0:DK, :]
                    )
                    nc.vector.tensor_copy(dsb[DK : DK + 1, :], pcs[1][DK : DK + 1, :])
                    qa = nc.sync if st % 2 == 0 else nc.gpsimd
                    qa.dma_start(
                        den_dram[st].rearrange("h (o f) -> h o f", o=1),
                        dsb[:].rearrange("(a g) f -> a g f", a=2)[:, 0:1, :],
                    )
                    dn = stage.tile(
                        [128, 2, 4], bf16, tag="dn", bufs=2, name=f"dn{st}"
                    )
                    rds[st] = dn
                    qa.dma_start(
                        dn[:], den_dram[st].rearrange("h (p j) -> p h j", p=128)
                    )

                units = []
                for q in range(4):
                    units.append(chain_quarter(0, q))
                    units.append(chain_quarter(1, q))
                units.append(evacuate)
                return units

            def div_recip(st):
                def unit():
                    dn = rds.pop(st)
                    rc = stage.tile(
                        [128, 2, 4], bf16, tag="rc", bufs=2, name=f"rc{st}"
                    )
                    with nc.allow_low_precision(
                        reason="bf16 softmax reciprocal within tolerance"
                    ):
                        nc.vector.reciprocal(rc[:], dn[:])
                    qb = nc.gpsimd if st % 2 == 0 else nc.sync
                    qb.dma_start(
                        rc_dram[st].rearrange("h (p j) -> p h j", p=128), rc[:]
                    )

                return unit

            def div_bcast(st):
                def unit():
                    # ctxT rows 0:64 = h1 (rc_dram[st,1]), 64:128 = h0
                    R0 = stage.tile([64, 512], bf16, tag="R0", bufs=2, name=f"R0_{st}")
                    R1 = stage.tile([128, 512], bf16, tag="R1", bufs=2, name=f"R1_{st}")
                    rds[("R", st)] = (R0, R1)
                    qa = nc.sync if st % 2 == 0 else nc.gpsimd
                    qb = nc.gpsimd if st % 2 == 0 else nc.sync
                    qa.dma_start(
                        R0[:],
                        rc_dram[st, 1, :]
                        .rearrange("(o f) -> o f", o=1)
                        .to_broadcast((64, 512)),
                    )
                    qb.dma_start(
                        R1[DK:128, :],
                        rc_dram[st, 0, :]
                        .rearrange("(o f) -> o f", o=1)
                        .to_broadcast((64, 512)),
                    )

                return unit

            def div_mul(st):
                def unit():
                    sl = slice(st * 512, (st + 1) * 512)
                    R0, R1 = rds.pop(("R", st))
                    nc.vector.tensor_mul(ctxT[0:DK, sl], ctxT[0:DK, sl], R0[:])
                    nc.vector.tensor_mul(
                        ctxT[DK:128, sl], ctxT[DK:128, sl], R1[DK:128, :]
                    )

                return unit

            def op_quad(st, j, engs=None):
                # 2 out-proj tiles (oc = 2j, 2j+1) -> one 128KB DMA
                def unit():
                    ob = stage.tile(
                        [128, 2, 512], bf16, tag="ob", bufs=2, name=f"ob{st}_{j}"
                    )
                    for k in range(2):
                        oc = j * 2 + k
                        po = op_ps.tile(
                            [128, 512], fp32, tag="op", name=f"po{st}_{oc}"
                        )
                        nc.tensor.matmul(
                            po[:],
                            wo_sb[:, oc, :],
                            ctxT[:, st * 512 : (st + 1) * 512],
                            start=True,
                            stop=True,
                        )
                        if engs is None:
                            nc.vector.tensor_copy(ob[:, k, :], po[:])
                        else:
                            engs[k % len(engs)](ob[:, k, :], po[:])
                    dq = nc.sync if (st + j) % 2 == 0 else nc.gpsimd
                    dq.dma_start(
                        out_r[:, j * 2 : j * 2 + 2, st * 512 : (st + 1) * 512],
                        ob[:],
                    )

                return unit

            def attn_block(b, sh, si, budgets, expS, inject=None):
                # both heads' scores into the two banks of one [128,1024]
                # PSUM tile (concurrent PE row-group tiles (0,0)/(64,0));
                # ONE FD=1024 exp covers both heads.
                s0 = b * S + sh * 1024 + si * 512
                for tt in range(16):
                    pump(budgets[tt])
                    if inject and tt in inject:
                        for u in inject[tt]:
                            u()
                    tb = slice(b * S + tt * 128, b * S + (tt + 1) * 128)
                    ps = sc_ps.tile([128, 1024], fp32, tag="sc", name="psAB")
                    nc.tensor.matmul(
                        ps[:, 0:512], krot[0:DK, tb], qrot[0:DK, s0 : s0 + 512],
                        start=True, stop=True,
                    )
                    nc.tensor.matmul(
                        ps[:, 512:1024],
                        krot[DK:128, tb],
                        qrot[DK:128, s0 : s0 + 512],
                        start=True, stop=True,
                    )
                    nc.scalar.activation(expS[:, tt, :], ps[:], Exp, scale=0.125)
                return expS

            # --- lead-in: stripe 0 K and Q explicit; chunks 2/3 follow
            # the stripe-0 swaps on sync/gpsimd, the rest on scalar.
            ps_k0 = proj_stripe(wk_sb, 0, "k")
            rope_stripe(ps_k0, bk_sb, 0, krot)
            ps_q0 = proj_stripe(wq_sb, 0, "q")
            rope_stripe(ps_q0, bq_sb, 0, qrot)
            chunk_dma(nc.sync, 2)
            chunk_dma(nc.gpsimd, 3)
            nc.scalar.dma_start(id_sb[:], ident)
            nc.scalar.dma_start(wv_sb[:], wv.rearrange("p (c m) -> p c m", m=128))
            for s in (4, 5, 6, 7):
                chunk_dma(nc.scalar, s)
            nc.scalar.dma_start(wo_sb[:], wo.rearrange("p (j m) -> p j m", m=128))

            # pump inventory, ordered against chunk arrival (chunks 4-7
            # land ~32/41/51/61us on the scalar queue) and emission
            # deadlines (producers emitted before consumers).
            for s in (1, 2, 3):
                work.append(qk_unit(wk_sb, bk_sb, s, krot, "k"))
            work.append(qk_unit(wq_sb, bq_sb, 1, qrot, "q"))
            for s in range(4):
                work.append(vproj_unit(s))
                for tt in range(4 * s, 4 * s + 4):
                    work.append(vtrans_unit(tt))
            for s in (2, 3):
                work.append(qk_unit(wq_sb, bq_sb, s, qrot, "q"))
            for s in (4, 5):
                work.append(qk_unit(wk_sb, bk_sb, s, krot, "k"))
            work.append(qk_unit(wq_sb, bq_sb, 4, qrot, "q"))
            for s in (4, 5):
                work.append(vproj_unit(s))
                for tt in range(4 * s, 4 * s + 4):
                    work.append(vtrans_unit(tt))
            for s in (6, 7):
                work.append(qk_unit(wk_sb, bk_sb, s, krot, "k"))
            for s in (6, 7):
                work.append(vproj_unit(s))
                for tt in range(4 * s, 4 * s + 4):
                    work.append(vtrans_unit(tt))
            for s in (5, 6, 7):
                work.append(qk_unit(wq_sb, bq_sb, s, qrot, "q"))

            blocks = [
                (b, sh, si) for b in range(B) for sh in range(2) for si in range(2)
            ]
            b0_budgets = [1] * 6 + [2] * 4 + [3] * 6
            prev = None
            cu7 = None
            for bi, (b, sh, si) in enumerate(blocks):
                if prev is not None:
                    work.extendleft(reversed(ctx_units(*prev)))
                expS = stage.tile(
                    [128, 16, 1024], bf16, tag="expS", bufs=2,
                    name=f"eS{b}{sh}{si}",
                )
                inject = None
                if bi == 7:
                    # last block: its own ctx chains ride its exp stream
                    cu7 = ctx_units(b, sh, si, expS)
                    inject = {4: cu7[0:2], 8: cu7[2:4], 12: cu7[4:6]}
                attn_block(
                    b, sh, si, b0_budgets if bi == 0 else [2] * 16, expS, inject
                )
                if bi >= 2:
                    pst = blocks[bi - 2]
                    stq = pst[0] * 4 + pst[1] * 2 + pst[2]
                    work.append(div_recip(stq))
                    work.append(div_bcast(stq))
                    work.append(div_mul(stq))
                    for j in range(4):
                        work.append(op_quad(stq, j))
                prev = (b, sh, si, expS)

            # drain: last block's ctx tail, remaining pump work, last stripes
            cu7[6]()
            cu7[7]()
            cu7[8]()
            while work:
                work.popleft()()
            drain_engs = [nc.vector.tensor_copy, nc.scalar.copy]
            for stq in (6, 7):
                div_recip(stq)()
                div_bcast(stq)()
                div_mul(stq)()
                for j in range(4):
                    op_quad(stq, j, drain_engs)()

    nc.compile()
    return nc


def _rope_tables():
    pos = np.arange(S, dtype=np.float64)
    inv_freq = np.exp(np.arange(0, DK, 2, dtype=np.float64) * (-np.log(10000.0) / DK))
    ang = pos[:, None] * inv_freq[None, :]  # [S, 32]
    cos_t = np.empty((128, S), dtype=np.float32)
    sin_t = np.empty((128, S), dtype=np.float32)
    c = np.cos(ang).astype(np.float32).T  # [32, S]
    s = np.sin(ang).astype(np.float32).T
    for blk in range(4):
        cos_t[blk * 32 : (blk + 1) * 32] = c
        sign = -1.0 if blk % 2 == 0 else 1.0
        sin_t[blk * 32 : (blk + 1) * 32] = sign * s
    return cos_t, sin_t


def _prep_w(w):
    # [1024, 128] column slice -> [128, 8*128] with the 1024-dim split into
    # 8 chunks of 128 on the partition axis (contiguous 2KB DMA lines)
    bf = ml_dtypes.bfloat16
    return np.ascontiguousarray(
        np.asarray(w, dtype=np.float32)
        .reshape(8, 128, 128)
        .transpose(1, 0, 2)
        .reshape(128, 8 * 128)
    ).astype(bf)


def _prep_inputs(inputs, Wq, bq, Wk, bk, Wv, bv, Wo):
    bf = ml_dtypes.bfloat16
    x2 = np.asarray(inputs, dtype=np.float32).reshape(NS, D)
    xt = np.ascontiguousarray(x2.T).astype(bf)
    cos_t, sin_t = _rope_tables()
    cos_b = cos_t.astype(bf)
    sin_b = sin_t.astype(bf)
    id_m = np.eye(128, dtype=np.float32).astype(bf)
    in_maps = []
    for c in range(NCORES):
        sl = slice(c * DPC, (c + 1) * DPC)
        # ctxT rows = [h1 dk | h0 dk] -> permute Wo rows to match
        wo_c = np.asarray(Wo[sl, :], dtype=np.float32)
        wo_c = np.concatenate([wo_c[DK:], wo_c[:DK]], axis=0)
        in_maps.append(
            {
                "xt": xt,
                "wq": _prep_w(Wq[:, sl]),
                "wk": _prep_w(Wk[:, sl]),
                "wv": _prep_w(Wv[:, sl]),
                "wo": np.ascontiguousarray(wo_c).astype(bf),
                "bq": np.ascontiguousarray(np.asarray(bq[sl], dtype=np.float32)).reshape(DPC, 1),
                "bk": np.ascontiguousarray(np.asarray(bk[sl], dtype=np.float32)).reshape(DPC, 1),
                "bv": np.ascontiguousarray(np.asarray(bv[sl], dtype=np.float32)).reshape(DPC, 1),
                "ident": id_m,
                "cos": cos_b,
                "sin": sin_b,
            }
        )
    return in_maps


def _get_nc():
    if "nc" not in _cache:
        _cache["nc"] = _build_nc()
    return _cache["nc"]


def run(inputs_dict, trace=False):
    """Build (cached), run on 8 cores, assemble full output. Returns
    (output fp32 [B,S,D], BassKernelResults)."""
    from concourse.bass_utils import run_bass_kernel_spmd

    nc = _get_nc()
    in_maps = _prep_inputs(
        inputs_dict["inputs"],
        inputs_dict["Wq"],
        inputs_dict["bq"],
        inputs_dict["Wk"],
        inputs_dict["bk"],
        inputs_dict["Wv"],
        inputs_dict["bv"],
        inputs_dict["Wo"],
    )
    res = run_bass_kernel_spmd(
        nc, in_maps, core_ids=list(range(NCORES)), trace=trace
    )
    acc = np.zeros((D, NS), dtype=np.float32)
    for r in res.results:
        acc += r["out"].astype(np.float32)
    out = acc.T.reshape(B, S, D) + np.asarray(inputs_dict["bo"], dtype=np.float32)
    return out.astype(np.float32), res


def kernel(**inputs):
    out, _ = run(inputs, trace=False)
    return out
